# revision 23
# baseline (speedup 1.0000x reference)
"""GIN message-passing network on 8 Trainium2 NeuronCores (Bass/Tile).

Strategy:
  - Nodes are split into 8 contiguous ranges at graph boundaries (so mean/sum
    pooling is core-local). Edges are owned by the core owning their dst node.
  - Each core keeps a full copy of node features h (node-major, [8*NPAD, 128]
    bf16) in HBM for gathering; an 8-core AllGather builds it from per-core
    slices (initially from the padded input x, then after each conv).
  - Aggregation (segment-sum over incoming edges) per 128-node dst block:
    dma_gather pulls h[src] rows for the block's edge chunks (int16 indices
    relative to a mid-table base so signed offsets span all rows), a DVE
    is_equal against an iota row builds each chunk's [128 edge x 128 node]
    one-hot (bf16), and PE matmuls Mg.T @ onehot accumulate agg^T in PSUM.
  - The GIN MLP runs transposed (features on partitions) so biases+ReLU fuse
    as per-partition scalar.activation; a PE transpose yields node-major
    h_new for the next round's gather table.
  - Pooling: one-hot graph matmul accumulated over all blocks, then the
    post-MLP, all on-device; host reassembles the [256, 128] output.
  - Per-core input bytes are kept minimal (~2 MB): the big tables (gather
    index replication, iota, graph one-hot, padded h0) are built on-device.
"""

import os
import numpy as np
import ml_dtypes

N = 50000
E = 800000
NF = 9
EMB = 128
HID = 256
L = 3
NUM_CONVS = 2
G = 256
NCORES = 8
P = 128


def _preprocess(x, edge_index, batch):
    """Host-side graph partitioning and edge-chunk layout."""
    gstart = np.searchsorted(batch, np.arange(G + 1))  # [G+1]

    # core graph splits balancing node counts
    gs = [0]
    for c in range(1, NCORES):
        t = (c * N) // NCORES
        i = int(np.searchsorted(gstart, t))
        if i > 0 and (i >= G + 1 or abs(int(gstart[i - 1]) - t) <= abs(int(gstart[i]) - t)):
            i -= 1
        i = max(gs[-1] + 1, min(i, G - (NCORES - c)))
        gs.append(i)
    gs.append(G)
    gs = np.array(gs, np.int64)
    ns = gstart[gs]  # node split points, ns[0]=0, ns[8]=N

    ncounts = np.diff(ns)
    NPAD = int(-(-ncounts.max() // P) * P)
    NB = NPAD // P
    ROWS = NCORES * NPAD

    # Two half-tables (split at SPLIT blocks, group-aligned): table A holds the
    # first SPLIT blocks of every core, table B the rest. Each half fits in
    # int16 (>=0) gather indices, and each is written by a single AllGather so
    # the A-half collective can overlap the conv's B-half compute.
    MLPG = 4
    SPLIT = max(MLPG, (NB // 2) // MLPG * MLPG)
    rowA = SPLIT * P
    ROWS_A = NCORES * rowA
    ROWS_B = NCORES * (NPAD - rowA)
    assert ROWS_A <= 32768 and ROWS_B <= 32768
    BASE = 0  # kept for the numpy scheme simulator

    node_ids = np.arange(N, dtype=np.int64)
    node_owner = np.searchsorted(ns, node_ids, side="right") - 1
    iloc = node_ids - ns[node_owner]
    in_b = iloc >= rowA
    # per-half local row id
    pid_loc = np.where(in_b, node_owner * (NPAD - rowA) + (iloc - rowA),
                       node_owner * rowA + iloc)

    src = np.asarray(edge_index[0], np.int64)
    dst = np.asarray(edge_index[1], np.int64)
    src_loc = pid_loc[src]
    src_half = in_b[src]
    dst_owner = node_owner[dst]

    # per-(core, block, half) counts -> shared chunk counts per half
    dl_all = dst - ns[dst_owner]
    blk_all = dl_all >> 7
    cntA = np.zeros((NCORES, NB), np.int64)
    cntB = np.zeros((NCORES, NB), np.int64)
    np.add.at(cntA, (dst_owner[~src_half], blk_all[~src_half]), 1)
    np.add.at(cntB, (dst_owner[src_half], blk_all[src_half]), 1)
    K_bA = np.maximum(-(-cntA.max(axis=0) // P), 1).astype(np.int64)
    K_bB = np.maximum(-(-cntB.max(axis=0) // P), 1).astype(np.int64)

    # group-major chunk stream: per group all A-chunks, then all B-chunks
    groups = []
    b0 = 0
    while b0 < NB:
        groups.append((b0, min(b0 + MLPG, NB)))
        b0 += MLPG
    baseA = np.zeros(NB, np.int64)
    baseB = np.zeros(NB, np.int64)
    CHT = 0
    for g0, g1 in groups:
        for b in range(g0, g1):
            baseA[b] = CHT
            CHT += int(K_bA[b])
        for b in range(g0, g1):
            baseB[b] = CHT
            CHT += int(K_bB[b])
    NIDX = CHT * P

    per_core = []
    for c in range(NCORES):
        flat_rel = np.zeros(NIDX, np.int32)  # pads: row 0 of the half-table
        flat_din = np.full(NIDX, -1.0, np.float32)
        for half, base_h in ((False, baseA), (True, baseB)):
            m = (dst_owner == c) & (src_half == half)
            sp = src_loc[m]
            blk = blk_all[m]
            din = (dl_all[m] & 127).astype(np.float32)
            order = np.argsort(blk, kind="stable")
            sp, din, blk = sp[order], din[order], blk[order]
            ccnt = np.bincount(blk, minlength=NB)
            first = np.concatenate([[0], np.cumsum(ccnt)])[:-1]
            rank = np.arange(len(sp)) - first[blk]
            pos = base_h[blk] * P + rank
            flat_rel[pos] = sp
            flat_din[pos] = din

        assert flat_rel.min() >= 0 and flat_rel.max() < 32768
        idx16 = flat_rel.astype(np.int16).reshape(-1, 16).T.copy()  # [16, NIDX/16]
        dstloc_np = flat_din.reshape(CHT, P).T.astype(
            ml_dtypes.bfloat16).copy()  # [128, CHT] bf16

        # per-core node ranges / pooling metadata
        ng = int(gs[c + 1] - gs[c])
        assert ng <= P
        bl = batch[ns[c]:ns[c + 1]] - gs[c]
        n_c = int(ncounts[c])
        # bl table for on-device one-hot build: bl_t[n, b] = graph-local id of
        # node b*128+n (or -1 for pads)
        bl_full = np.full(NPAD, -1.0, np.float32)
        bl_full[:n_c] = bl
        bl_t = bl_full.reshape(NB, P).T.astype(ml_dtypes.bfloat16).copy()  # [128, NB]
        # host-side ohg (only used by the numpy scheme simulator in test.py)
        ohg = np.zeros((NPAD, P), np.float32)
        ohg[np.arange(n_c), bl] = 1.0
        ohg_t = ohg.reshape(NB, P, P).transpose(1, 0, 2).reshape(P, NB * P).astype(ml_dtypes.bfloat16)
        cnts = np.bincount(bl, minlength=P)[:P]
        invc = np.zeros((P, 1), np.float32)
        invc[:ng, 0] = 1.0 / np.maximum(cnts[:ng], 1)

        # compact transposed x slice: [16, NPAD] bf16, rows 0..8 = features
        xT = np.zeros((16, NPAD), ml_dtypes.bfloat16)
        xT[:NF, :n_c] = np.asarray(x[ns[c]:ns[c + 1]]).T.astype(ml_dtypes.bfloat16)

        per_core.append(dict(idx=idx16, dstloc=dstloc_np, bl=bl_t, xT=xT,
                             ohg=ohg_t, invc=invc, ng=ng, n_c=n_c))

    # initial h (only used by the numpy scheme simulator in test.py);
    # rows [0, ROWS_A) = table A, [ROWS_A, ROWS) = table B
    h0_full = np.zeros((ROWS, EMB), np.float32)
    for c in range(NCORES):
        n_c = int(ncounts[c])
        hc = np.zeros((NPAD, EMB), np.float32)
        hc[:n_c, :NF] = x[ns[c]:ns[c + 1]]
        h0_full[c * rowA:(c + 1) * rowA] = hc[:rowA]
        h0_full[ROWS_A + c * (NPAD - rowA):
                ROWS_A + (c + 1) * (NPAD - rowA)] = hc[rowA:]

    geom = dict(NPAD=NPAD, NB=NB, ROWS=ROWS, BASE=BASE, K_bA=K_bA, K_bB=K_bB,
                baseA=baseA, baseB=baseB, CHT=CHT, NIDX=NIDX, ns=ns, gs=gs,
                SPLIT=SPLIT, rowA=rowA, ROWS_A=ROWS_A, ROWS_B=ROWS_B)
    return geom, per_core, h0_full


def _pack_weights(gin_w1, gin_b1, gin_w2, gin_b2, post_w1, post_b1, post_w2,
                  post_b2):
    w1 = np.concatenate([gin_w1[l] for l in range(L)], axis=1).astype(
        ml_dtypes.bfloat16)  # [128, 768]
    w2 = np.concatenate(
        [gin_w2[l][h * P:(h + 1) * P, :] for l in range(L) for h in (0, 1)],
        axis=1).astype(ml_dtypes.bfloat16)  # [128, 768]
    b1 = np.stack([gin_b1[l][h * P:(h + 1) * P] for l in range(L) for h in (0, 1)],
                  axis=1)  # [128, 6]
    b2 = np.stack([gin_b2[l] for l in range(L)], axis=1)  # [128, 3]
    pw1 = np.concatenate(
        [post_w1[kc * P:(kc + 1) * P, mh * P:(mh + 1) * P]
         for kc in (0, 1) for mh in (0, 1)], axis=1)  # [128, 512]
    pw2 = np.concatenate([post_w2[kc * P:(kc + 1) * P, :] for kc in (0, 1)],
                         axis=1)  # [128, 256]
    pb1 = np.stack([post_b1[mh * P:(mh + 1) * P] for mh in (0, 1)], axis=1)
    pb2 = post_b2[:, None]
    return dict(w1=w1, w2=w2, b1=b1, b2=b2, pw1=pw1, pw2=pw2, pb1=pb1, pb2=pb2)


def _build_program(geom, n_convs, reps=1):
    import concourse.bass as bass
    import concourse.bacc as bacc
    import concourse.tile as tile
    import concourse.mybir as mybir
    from concourse.masks import make_identity

    F32 = mybir.dt.float32
    BF16 = mybir.dt.bfloat16
    I16 = mybir.dt.int16
    Relu = mybir.ActivationFunctionType.Relu

    NPAD, NB, ROWS = geom["NPAD"], geom["NB"], geom["ROWS"]
    CHT, NIDX = geom["CHT"], geom["NIDX"]
    K_bA, K_bB = geom["K_bA"], geom["K_bB"]
    baseA, baseB = geom["baseA"], geom["baseB"]
    SPLIT, rowA = geom["SPLIT"], geom["rowA"]
    ROWS_A, ROWS_B = geom["ROWS_A"], geom["ROWS_B"]

    n_queues = int(os.environ.get("GNN_GQ", "4"))
    OHG = int(os.environ.get("GNN_OHG", "8"))  # onehot chunks per DVE op
    MLPG = 4     # 128-node blocks per MLP group (moving dim 512)

    ndev = int(os.environ.get("GNN_NDEV", str(NCORES)))
    no_cc = os.environ.get("GNN_NO_CC", "0") == "1"
    nc = bacc.Bacc("TRN2", target_bir_lowering=False, debug=False,
                   enable_asserts=True, num_devices=ndev,
                   num_swdge_queues=4,
                   dynamic_dma_scratch_size=int(os.environ.get(
                       "GNN_DMA_SCRATCH", "32768")))

    t_xT = nc.dram_tensor("t_xT", [16, NPAD], BF16, kind="ExternalInput")
    t_idx16 = nc.dram_tensor("t_idx16", [16, NIDX // 16], I16, kind="ExternalInput")
    t_dstloc = nc.dram_tensor("t_dstloc", [P, CHT], BF16, kind="ExternalInput")
    t_bl = nc.dram_tensor("t_bl", [P, NB], BF16, kind="ExternalInput")
    t_invc = nc.dram_tensor("t_invc", [P, 1], F32, kind="ExternalInput")
    t_w1 = nc.dram_tensor("t_w1", [P, L * 2 * P], BF16, kind="ExternalInput")
    t_w2 = nc.dram_tensor("t_w2", [P, L * 2 * P], BF16, kind="ExternalInput")
    t_b1 = nc.dram_tensor("t_b1", [P, L * 2], F32, kind="ExternalInput")
    t_b2 = nc.dram_tensor("t_b2", [P, L], F32, kind="ExternalInput")
    t_pw1 = nc.dram_tensor("t_pw1", [P, 4 * P], F32, kind="ExternalInput")
    t_pw2 = nc.dram_tensor("t_pw2", [P, 2 * P], F32, kind="ExternalInput")
    t_pb1 = nc.dram_tensor("t_pb1", [P, 2], F32, kind="ExternalInput")
    t_pb2 = nc.dram_tensor("t_pb2", [P, 1], F32, kind="ExternalInput")
    o_outT = nc.dram_tensor("o_outT", [P, P], F32, kind="ExternalOutput")

    # MLP block groups
    groups = []
    b0 = 0
    while b0 < NB:
        groups.append((b0, min(b0 + MLPG, NB)))
        b0 += MLPG

    R = reps * n_convs

    with tile.TileContext(nc) as tc:
        with tc.tile_pool(name="const", bufs=1) as cp, \
             tc.tile_pool(name="mgp", bufs=int(os.environ.get("GNN_MGB", "4"))) as mgp, \
             tc.tile_pool(name="work", bufs=2) as wp, \
             tc.tile_pool(name="oh", bufs=8) as ohp, \
             tc.tile_pool(name="psA", bufs=2, space="PSUM") as psA, \
             tc.tile_pool(name="psT", bufs=1, space="PSUM") as psT_pool, \
             tc.tile_pool(name="psB", bufs=2, space="PSUM") as psB, \
             tc.tile_pool(name="psM", bufs=1, space="PSUM") as psM, \
             tc.tile_pool(name="psC", bufs=1, space="PSUM") as psC, \
             tc.tile_pool(name="dram", bufs=1, space="DRAM") as dram:

            idx_sb = cp.tile([P, NIDX // 16], I16)
            dstloc_sb = cp.tile([P, CHT], BF16)
            iota_sb = cp.tile([P, OHG * P], BF16)
            ohg_sb = cp.tile([P, NB * P], BF16)
            bl_sb = cp.tile([P, NB], BF16)
            invc_sb = cp.tile([P, 1], F32)
            w1_sb = cp.tile([P, L * 2 * P], BF16)
            w2_sb = cp.tile([P, L * 2 * P], BF16)
            b1_sb = cp.tile([P, L * 2], F32)
            b2_sb = cp.tile([P, L], F32)
            pw1_sb = cp.tile([P, 4 * P], F32)
            pw2_sb = cp.tile([P, 2 * P], F32)
            pb1_sb = cp.tile([P, 2], F32)
            pb2_sb = cp.tile([P, 1], F32)
            ident = cp.tile([P, P], F32)
            for sb_t, dr_t in [(dstloc_sb, t_dstloc), (bl_sb, t_bl),
                               (invc_sb, t_invc), (w1_sb, t_w1), (w2_sb, t_w2),
                               (b1_sb, t_b1), (b2_sb, t_b2), (pw1_sb, t_pw1),
                               (pw2_sb, t_pw2), (pb1_sb, t_pb1),
                               (pb2_sb, t_pb2)]:
                nc.sync.dma_start(sb_t[:], dr_t[:])
            make_identity(nc, ident[:])

            # gather-index replication 16 -> 128 partitions (8x, on device)
            nc.sync.dma_start(idx_sb[0:16, :], t_idx16[:])
            nc.sync.dma_start(idx_sb[16:32, :], idx_sb[0:16, :])
            nc.sync.dma_start(idx_sb[32:64, :], idx_sb[0:32, :])
            nc.sync.dma_start(idx_sb[64:128, :], idx_sb[0:64, :])

            # iota row table (0..127 repeated OHG times, every partition)
            nc.gpsimd.iota(iota_sb[:], [[0, OHG], [1, P]],
                           channel_multiplier=0,
                           allow_small_or_imprecise_dtypes=True)

            # pooling one-hot from graph labels (on device)
            for j0 in range(0, NB, OHG):
                kn = min(OHG, NB - j0)
                nc.vector.tensor_tensor(
                    out=ohg_sb[:, j0 * P:(j0 + kn) * P].rearrange(
                        "p (a b) -> p a b", b=P),
                    in0=iota_sb[:, :kn * P].rearrange("p (a b) -> p a b", b=P),
                    in1=bl_sb[:, j0:j0 + kn].to_broadcast([P, kn, P]),
                    op=mybir.AluOpType.is_equal)

            # persistent feature-major h (ping-pong), bf16
            hT0 = cp.tile([P, NPAD], BF16)
            hT1 = cp.tile([P, NPAD], BF16)
            hT_pp = [hT0, hT1]
            ident_bf = cp.tile([P, P], BF16)
            make_identity(nc, ident_bf[:])
            nc.vector.memset(hT0[:], 0.0)
            nc.sync.dma_start(hT0[0:16, :], t_xT[:])

            hnew0 = dram.tile([NPAD, EMB], BF16)
            hnew1 = dram.tile([NPAD, EMB], BF16)
            hnew_pp = [hnew0, hnew1]
            hinit = dram.tile([NPAD, EMB], BF16)
            hfA_cv = [dram.tile([ROWS_A, EMB], BF16, addr_space="Shared",
                                name=f"hfa{i}") for i in range(R)]
            hfB_cv = [dram.tile([ROWS_B, EMB], BF16, addr_space="Shared",
                                name=f"hfb{i}") for i in range(R)]

            # initial node-major h0 + AllGather of the shared gather table
            for b in range(NB):
                bs = slice(b * P, (b + 1) * P)
                psT = psT_pool.tile([P, P], BF16, space="PSUM", tag="tp",
                                    name=f"tp_init_{b}")
                nc.tensor.transpose(out=psT[:], in_=hT0[:, bs],
                                    identity=ident_bf[:])
                hnode = wp.tile([P, P], BF16, tag="hnode", name=f"hn_init_{b}")
                nc.scalar.copy(out=hnode[:], in_=psT[:])
                nc.sync.dma_start(hinit[bs, :], hnode[:])
            if not no_cc:
                nc.gpsimd.collective_compute(
                    "AllGather", mybir.AluOpType.bypass,
                    replica_groups=[list(range(NCORES))],
                    ins=[hinit[0:rowA, :].opt()],
                    outs=[hfA_cv[0].opt()])
                nc.gpsimd.collective_compute(
                    "AllGather", mybir.AluOpType.bypass,
                    replica_groups=[list(range(NCORES))],
                    ins=[hinit[rowA:, :].opt()],
                    outs=[hfB_cv[0].opt()])

            psum_pool = psC.tile([P, P], F32, space="PSUM", tag="pool")

            for gc in range(R):
                r, c = divmod(gc, n_convs)
                l = min(c // NUM_CONVS, L - 1)
                hT_cur = hT_pp[gc % 2]
                hT_nxt = hT_pp[(gc + 1) % 2]
                srcA, srcB = hfA_cv[gc], hfB_cv[gc]
                last = gc == R - 1

                for gi, (g0, g1) in enumerate(groups):
                    # aggregation: one gather per src half-table (A-half can
                    # start as soon as the previous conv's A AllGather lands)
                    ob0 = int(baseA[g0])
                    gAn = int(baseB[g0]) - ob0
                    kbsum = int(baseB[g1 - 1] + K_bB[g1 - 1]) - ob0
                    mg = mgp.tile([P, kbsum, P], BF16, tag="mg",
                                  name=f"mg_{gc}_{g0}")
                    skip_gather = os.environ.get("GNN_SKIP_GATHER", "0") == "1"
                    if skip_gather:  # timing-only: stand-in write, no DMA
                        nc.vector.memset(mg[:, 0, :], 0.0)
                    for si, (c0, c1, src_t) in enumerate(
                            ((0, gAn, srcA), (gAn, kbsum, srcB))):
                        if c0 >= c1 or skip_gather:
                            continue
                        nc.gpsimd.dma_gather(
                            out_ap=mg[:, c0:c1, :],
                            in_ap=src_t[0:, :],
                            idxs_ap=idx_sb[:, (ob0 + c0) * 8:(ob0 + c1) * 8],
                            num_idxs=(c1 - c0) * P,
                            num_idxs_reg=(c1 - c0) * P,
                            elem_size=EMB,
                            single_packet=False,
                            queue_num=(2 * gi + si) % n_queues,
                        )
                    # onehot chunks for the whole group (block-agnostic)
                    ohts = []  # global-chunk c -> (tile, pos within tile)
                    for j0 in range(0, kbsum, OHG):
                        kn = min(OHG, kbsum - j0)
                        oht = ohp.tile([P, OHG, P], BF16, tag="oh",
                                       name=f"oh_{gc}_{g0}_{j0}")
                        nc.vector.tensor_tensor(
                            out=oht[:, :kn, :],
                            in0=iota_sb[:, :kn * P].rearrange(
                                "p (a b) -> p a b", b=P),
                            in1=dstloc_sb[:, ob0 + j0:ob0 + j0 + kn]
                                .to_broadcast([P, kn, P]),
                            op=mybir.AluOpType.is_equal)
                        for kk in range(kn):
                            ohts.append((oht, kk))
                    aggT = wp.tile([P, 512], BF16, tag="aggT",
                                   name=f"aggT_{gc}_{g0}")
                    for b in range(g0, g1):
                        tot = int(K_bA[b]) + int(K_bB[b])
                        psumA = psA.tile([P, P], F32, space="PSUM", tag="agg",
                                         name=f"agg_{gc}_{b}")
                        ki = 0
                        for cbase, kn in ((int(baseA[b]) - ob0, int(K_bA[b])),
                                          (int(baseB[b]) - ob0, int(K_bB[b]))):
                            for k in range(kn):
                                oht, kk = ohts[cbase + k]
                                nc.tensor.matmul(out=psumA[:],
                                                 lhsT=mg[:, cbase + k, :],
                                                 rhs=oht[:, kk, :],
                                                 start=(ki == 0),
                                                 stop=(ki == tot - 1))
                                ki += 1
                        lb = b - g0
                        nc.scalar.copy(out=aggT[:, lb * P:(lb + 1) * P],
                                       in_=psumA[:])
                    # grouped MLP: moving dim = 128 * (g1 - g0)
                    gw = (g1 - g0) * P
                    gsl = slice(g0 * P, g0 * P + gw)
                    z1 = []
                    for mh in range(2):
                        ps1 = psB.tile([P, 512], F32, space="PSUM", tag="mm1",
                                       name=f"mm1_{gc}_{g0}_{mh}")
                        nc.tensor.matmul(
                            out=ps1[:, :gw],
                            lhsT=w1_sb[:, (l * 2 + mh) * P:(l * 2 + mh + 1) * P],
                            rhs=aggT[:, :gw], start=True, stop=False)
                        nc.tensor.matmul(
                            out=ps1[:, :gw],
                            lhsT=w1_sb[:, (l * 2 + mh) * P:(l * 2 + mh + 1) * P],
                            rhs=hT_cur[:, gsl], start=False, stop=True)
                        z1t = wp.tile([P, 512], BF16, tag=f"z1_{mh}",
                                      name=f"z1_{gc}_{g0}_{mh}")
                        nc.scalar.activation(
                            out=z1t[:, :gw], in_=ps1[:, :gw], func=Relu,
                            bias=b1_sb[:, l * 2 + mh:l * 2 + mh + 1])
                        z1.append(z1t)
                    ps2 = psM.tile([P, 512], F32, space="PSUM", tag="mm2",
                                   name=f"mm2_{gc}_{g0}")
                    for mh in range(2):
                        nc.tensor.matmul(
                            out=ps2[:, :gw],
                            lhsT=w2_sb[:, (l * 2 + mh) * P:(l * 2 + mh + 1) * P],
                            rhs=z1[mh][:, :gw], start=(mh == 0), stop=(mh == 1))
                    nc.scalar.activation(out=hT_nxt[:, gsl], in_=ps2[:, :gw],
                                         func=Relu, bias=b2_sb[:, l:l + 1])
                    # A-half AllGather as soon as its blocks are stored
                    if g0 == SPLIT and not last and not no_cc:
                        nc.gpsimd.collective_compute(
                            "AllGather", mybir.AluOpType.bypass,
                            replica_groups=[list(range(NCORES))],
                            ins=[hnew_pp[gc % 2][0:rowA, :].opt()],
                            outs=[hfA_cv[gc + 1].opt()])
                    # node-major h_new per block (for allgather / pooling)
                    for b in range(g0, g1):
                        bs = slice(b * P, (b + 1) * P)
                        psT = psT_pool.tile([P, P], BF16, space="PSUM", tag="tp",
                                            name=f"tp_{gc}_{b}")
                        nc.tensor.transpose(out=psT[:], in_=hT_nxt[:, bs],
                                            identity=ident_bf[:])
                        if not last:
                            hnode = wp.tile([P, P], BF16, tag="hnode",
                                            name=f"hn_{gc}_{b}")
                            nc.scalar.copy(out=hnode[:], in_=psT[:])
                            nc.sync.dma_start(hnew_pp[gc % 2][bs, :], hnode[:])
                        else:
                            hnode = wp.tile([P, P], BF16, tag="hnode",
                                            name=f"hn_{gc}_{b}")
                            nc.scalar.copy(out=hnode[:], in_=psT[:])
                            nc.tensor.matmul(out=psum_pool[:],
                                             lhsT=ohg_sb[:, bs], rhs=hnode[:],
                                             start=(b == 0), stop=(b == NB - 1),
                                             skip_group_check=True)
                if not last and not no_cc:
                    nc.gpsimd.collective_compute(
                        "AllGather", mybir.AluOpType.bypass,
                        replica_groups=[list(range(NCORES))],
                        ins=[hnew_pp[gc % 2][rowA:, :].opt()],
                        outs=[hfB_cv[gc + 1].opt()])

            # pooling epilogue
            sums_sb = cp.tile([P, P], F32)
            means_sb = cp.tile([P, P], F32)
            nc.vector.tensor_copy(out=sums_sb[:], in_=psum_pool[:])
            nc.vector.tensor_scalar(out=means_sb[:], in0=psum_pool[:],
                                    scalar1=invc_sb[:, 0:1], scalar2=None,
                                    op0=mybir.AluOpType.mult)
            psTs = psT_pool.tile([P, P], F32, space="PSUM", tag="tp")
            nc.tensor.transpose(out=psTs[:], in_=sums_sb[:], identity=ident[:])
            sT = cp.tile([P, P], F32)
            nc.scalar.copy(out=sT[:], in_=psTs[:])
            psTm = psT_pool.tile([P, P], F32, space="PSUM", tag="tp")
            nc.tensor.transpose(out=psTm[:], in_=means_sb[:], identity=ident[:])
            mT = cp.tile([P, P], F32)
            nc.scalar.copy(out=mT[:], in_=psTm[:])

            z1p = []
            for mh in range(2):
                ps3 = psB.tile([P, 512], F32, space="PSUM", tag="mm1")
                nc.tensor.matmul(out=ps3[:, :P],
                                 lhsT=pw1_sb[:, (0 * 2 + mh) * P:(0 * 2 + mh + 1) * P],
                                 rhs=sT[:], start=True, stop=False)
                nc.tensor.matmul(out=ps3[:, :P],
                                 lhsT=pw1_sb[:, (1 * 2 + mh) * P:(1 * 2 + mh + 1) * P],
                                 rhs=mT[:], start=False, stop=True)
                z1t = cp.tile([P, P], F32, name=f"z1p_{mh}")
                nc.scalar.activation(out=z1t[:], in_=ps3[:, :P], func=Relu,
                                     bias=pb1_sb[:, mh:mh + 1])
                z1p.append(z1t)
            ps4 = psM.tile([P, 512], F32, space="PSUM", tag="mm2")
            for kc in range(2):
                nc.tensor.matmul(out=ps4[:, :P], lhsT=pw2_sb[:, kc * P:(kc + 1) * P],
                                 rhs=z1p[kc][:], start=(kc == 0), stop=(kc == 1))
            out_sb = cp.tile([P, P], F32)
            nc.vector.tensor_scalar(out=out_sb[:], in0=ps4[:, :P],
                                    scalar1=pb2_sb[:, 0:1], scalar2=None,
                                    op0=mybir.AluOpType.add)
            nc.sync.dma_start(o_outT[:], out_sb[:])

    nc.compile()
    return nc


def _make_in_maps(geom, per_core, w):
    in_maps = []
    for c in range(NCORES):
        pc = per_core[c]
        in_maps.append({
            "t_xT": pc["xT"], "t_idx16": pc["idx"], "t_dstloc": pc["dstloc"],
            "t_bl": pc["bl"], "t_invc": pc["invc"], "t_w1": w["w1"],
            "t_w2": w["w2"], "t_b1": w["b1"], "t_b2": w["b2"],
            "t_pw1": w["pw1"], "t_pw2": w["pw2"], "t_pb1": w["pb1"],
            "t_pb2": w["pb2"],
        })
    return in_maps


def kernel(**inputs):
    x = np.asarray(inputs["x"], np.float32)
    edge_index = np.asarray(inputs["edge_index"], np.int64)
    batch = np.asarray(inputs["batch"], np.int64)
    gin_w1 = np.asarray(inputs["gin_w1"], np.float32)
    gin_b1 = np.asarray(inputs["gin_b1"], np.float32)
    gin_w2 = np.asarray(inputs["gin_w2"], np.float32)
    gin_b2 = np.asarray(inputs["gin_b2"], np.float32)
    post_w1 = np.asarray(inputs["post_w1"], np.float32)
    post_b1 = np.asarray(inputs["post_b1"], np.float32)
    post_w2 = np.asarray(inputs["post_w2"], np.float32)
    post_b2 = np.asarray(inputs["post_b2"], np.float32)

    geom, per_core, h0_full = _preprocess(x, edge_index, batch)
    w = _pack_weights(gin_w1, gin_b1, gin_w2, gin_b2, post_w1, post_b1,
                      post_w2, post_b2)

    n_convs = int(os.environ.get("GNN_CONVS", L * NUM_CONVS))
    nc = _build_program(geom, n_convs, reps=int(os.environ.get('GNN_REPS', '1')))

    in_maps = _make_in_maps(geom, per_core, w)

    from concourse.bass_utils import run_bass_kernel_spmd
    trace = os.environ.get("GNN_TRACE", "0") == "1"
    res = run_bass_kernel_spmd(nc, in_maps, core_ids=list(range(NCORES)),
                               trace=trace)
    if trace:
        print(f"HW exec time: {res.exec_time_ns} ns")
        kernel.last_results = res

    gs = geom["gs"]
    out = np.zeros((G, EMB), np.float32)
    for c in range(NCORES):
        outT = res.results[c]["o_outT"]  # [emb, graph slots]
        ng = per_core[c]["ng"]
        out[gs[c]:gs[c] + ng] = outT[:, :ng].T
    return out
